# revision 36
# baseline (speedup 1.0000x reference)
"""Distributed AttentionHead kernel for 8 TRN2 NeuronCores.

Problem: qkv = x @ w.T ; q,k,v = split(qkv); scores[i,j] = k_i.q_j/sqrt(E),
mask keeps j >= i; out = softmax(scores) @ v.   B,N,H,E = 4,2048,1024,1024.

Sharding: core c = 2*b + s handles batch b; it owns the 8 row-tiles
{128*(2*lt+s) : lt in 0..7} (parity interleave => every core's attention
loop has j-extents (16,14,12,...,2) tiles => a single uniform SPMD graph).
Masks that differ between even/odd cores are passed as input *data*,
keeping the graph identical on all cores (required: collectives-free SPMD).

Algebraic restructure (saves ~2x projection FLOPs vs materializing q/v):
  scores = K . (Wq x^T) = (K Wq) . x^T           -> T = K Wq, own rows only
  out    = P (X Wv^T)   = (P X) Wv^T             -> U = P X,  own rows only
so no full-batch q or v is ever computed; every GEMM is own-row-sized
except the (mask-skipped) attention contractions themselves.

Per-core work: k^T proj 2.15 GF + T 2.15 + scores ~2.4 + U ~2.4 + out 2.15
= ~11.3 GF.  Compute dtype: bf16 operands (cast during DMA), fp32 accum.
"""
import os
import sys

sys.path.insert(0, "/opt/trn_rl_repo")

import numpy as np
import ml_dtypes

import concourse.mybir as mybir
from concourse import bacc
from concourse.tile import TileContext
from concourse.bass_utils import run_bass_kernel_spmd

B, N, H, E = 4, 2048, 1024, 1024
O3 = 3 * E
NT = N // 128          # 16 row tiles per batch
LT = 8                 # row tiles owned per core
BF = mybir.dt.bfloat16
F32 = mybir.dt.float32

_CACHE = {}
LAST_RESULT = None


def _build():
    nc = bacc.Bacc("TRN2", target_bir_lowering=False, debug=False, num_devices=8)

    xT_ext = nc.dram_tensor("xT", [H, N], F32, kind="ExternalInput")
    xn_ext = nc.dram_tensor("xn", [N, H], F32, kind="ExternalInput")
    wQ_ext = nc.dram_tensor("wQ", [E, H], F32, kind="ExternalInput")
    wT_ext = nc.dram_tensor("wT", [H, O3], F32, kind="ExternalInput")
    am_ext = nc.dram_tensor("amask", [128, 256], F32, kind="ExternalInput")
    id_ext = nc.dram_tensor("ident", [128, 128], BF, kind="ExternalInput")
    out_ext = nc.dram_tensor("out", [LT, 128, 1024], F32, kind="ExternalOutput")

    xT_r = xT_ext.rearrange("(hs p) n -> p hs n", p=128)
    xn_r = xn_ext.rearrange("(jt p) h -> p jt h", p=128)
    wQ_r = wQ_ext.rearrange("(es p) h -> p es h", p=128)
    wT_r = wT_ext.rearrange("(hs p) o -> p hs o", p=128)

    with TileContext(nc) as tc:
        with (
            tc.tile_pool(name="consts", bufs=1) as consts,
            tc.tile_pool(name="wts", bufs=4) as wts,
            tc.tile_pool(name="bigx", bufs=1) as bigx,
            tc.tile_pool(name="qkv", bufs=1) as qkv,
            tc.tile_pool(name="pbuf", bufs=1) as pbuf,
            tc.tile_pool(name="pts", bufs=6) as ptsp,
            tc.tile_pool(name="ubuf", bufs=3) as ubuf,
            tc.tile_pool(name="utb", bufs=2) as utb,
            tc.tile_pool(name="outb", bufs=2) as outb,
            tc.tile_pool(name="smalls", bufs=3) as smalls,
            tc.tile_pool(name="acc", bufs=4, space="PSUM") as accp,
            tc.tile_pool(name="sc", bufs=2, space="PSUM") as scp,
            tc.tile_pool(name="tp", bufs=2, space="PSUM") as tpp,
        ):
            ident = consts.tile([128, 128], BF)
            nc.sync.dma_start(out=ident, in_=id_ext[:, :])
            am_sb = consts.tile([128, 256], F32)
            nc.sync.dma_start(out=am_sb, in_=am_ext[:, :])

            # Warm-up matmuls: keep the PE busy while the first inputs load so
            # the HAM clock gate reaches 8/8 before real work arrives, instead
            # of paying the cold 1.2 GHz ramp on the first projection.
            wu_sb = consts.tile([128, 512], BF)
            nc.vector.memset(wu_sb, 0.0)
            wu_lhs = consts.tile([128, 128], BF)
            nc.vector.memset(wu_lhs, 0.0)
            wu_ps = accp.tile([128, 512], F32, tag="acc", name="wu_ps")
            for r in range(18):
                nc.tensor.matmul(wu_ps, wu_lhs, wu_sb, start=True, stop=True)

            # k-projection inputs, ordered for earliest possible PE start:
            # first wT chunk first (only 1MB), then xTo slab-by-slab so the
            # first psum group can begin as soon as slab 0 lands.
            wtc_pre = {}
            wtc = wts.tile([128, 8, 256], BF, tag="wtc", name="wtc4")
            nc.gpsimd.dma_start(out=wtc, in_=wT_r[:, :, 256 * 4:256 * 4 + 256])
            wtc_pre[4] = wtc

            xT_sb = bigx.tile([128, 8, N], BF)
            for hs in range(8):
                nc.gpsimd.dma_start(
                    out=xT_sb[:, hs, 0:N // 2], in_=xT_r[:, hs, 0:N // 2]
                )

            for oc in (5, 6, 7):
                wtc = wts.tile([128, 8, 256], BF, tag="wtc", name=f"wtc{oc}")
                nc.gpsimd.dma_start(out=wtc, in_=wT_r[:, :, 256 * oc:256 * oc + 256])
                wtc_pre[oc] = wtc

            wQ_sb = bigx.tile([128, 8, H], BF)
            nc.gpsimd.dma_start(out=wQ_sb, in_=wQ_r)

            nc.gpsimd.dma_start(out=xT_sb[:, :, N // 2:N], in_=xT_r[:, :, N // 2:N])

            xn_sb = qkv.tile([128, NT, H], BF, tag="xn", name="xn_sb")
            nc.gpsimd.dma_start(out=xn_sb, in_=xn_r)
            xn = [xn_sb[:, t, :] for t in range(NT)]

            wvT_sb = bigx.tile([128, 8, E], BF)
            nc.gpsimd.dma_start(out=wvT_sb, in_=wT_r[:, :, 2 * E:3 * E])

            kT = [qkv.tile([128, N // 2], BF, tag=f"kT{e}", name=f"kT{e}") for e in range(8)]
            TT = [qkv.tile([128, N // 2], BF, tag=f"TT{h}", name=f"TT{h}") for h in range(8)]

            # ---------------- k^T projection (own rows) ----------------
            for oc in (4, 5, 6, 7):
                wtc = wtc_pre[oc]
                for n0 in range(0, N // 2, 512):
                    for ot in range(2):
                        et = 2 * (oc - 4) + ot
                        ps = accp.tile([128, 512], F32, tag="acc", name="ps_k")
                        for hs in range(8):
                            nc.tensor.matmul(
                                ps,
                                wtc[:, hs, 128 * ot:128 * ot + 128],
                                xT_sb[:, hs, n0:n0 + 512],
                                start=hs == 0,
                                stop=hs == 7,
                            )
                        nc.scalar.copy(out=kT[et][:, n0:n0 + 512], in_=ps)

            # ---------------- T^T = (K Wq)^T = Wq^T K^T ----------------
            for i0 in range(0, N // 2, 512):
                for ht in range(8):
                    ps = accp.tile([128, 512], F32, tag="acc", name="ps_t")
                    for es in range(8):
                        nc.tensor.matmul(
                            ps,
                            wQ_sb[:, es, 128 * ht:128 * ht + 128],
                            kT[es][:, i0:i0 + 512],
                            start=es == 0,
                            stop=es == 7,
                        )
                    nc.vector.tensor_copy(out=TT[ht][:, i0:i0 + 512], in_=ps)

            # ---------------- attention ----------------
            for li in range(LT):
                nch = 8 - li          # 256-wide score chunks
                nj = NT - 2 * li      # 128-wide j tiles
                p = pbuf.tile([128, 256 * nch], BF, tag=f"p{li}", name=f"p{li}")
                asum = smalls.tile([128, 8], F32, tag="asum", name=f"asum{li}")
                for ch in range(nch):
                    g = li + ch
                    ps = scp.tile([128, 256], F32, tag="sc", name=f"ps_s{li}_{ch}")
                    for hs in range(8):
                        rhs = xT_sb[:, hs, :].rearrange(
                            "p (two g c) -> p two g c", two=2, c=128
                        )[:, :, g, :]
                        nc.tensor.matmul(
                            ps,
                            TT[hs][:, 128 * li:128 * li + 128],
                            rhs,
                            start=hs == 0,
                            stop=hs == 7,
                        )
                    if ch == 0:
                        nc.vector.tensor_add(ps, ps, am_sb)
                    nc.scalar.activation(
                        out=p[:, 256 * ch:256 * ch + 256],
                        in_=ps,
                        func=mybir.ActivationFunctionType.Exp,
                        scale=float(1.0 / np.sqrt(E)),
                        accum_out=asum[:, ch:ch + 1],
                    )
                ut = utb.tile([128, 8, 128], BF, tag="ut", name=f"ut{li}")
                if li < 5:
                    # U = P X  (f32 accum in PSUM, bf16 out), then transpose U
                    pv0 = accp.tile([128, 512], F32, tag="acc", name=f"pv0_{li}")
                    pv1 = accp.tile([128, 512], F32, tag="acc", name=f"pv1_{li}")
                    for u in range(nj):
                        tp = tpp.tile([128, 128], BF, tag="tp", name=f"tp{li}_{u}")
                        nc.tensor.transpose(tp, p[:, 128 * u:128 * u + 128], ident)
                        pt = ptsp.tile([128, 128], BF, tag="pts", name=f"pt{li}_{u}")
                        nc.vector.tensor_copy(out=pt, in_=tp)
                        jt = (li + u // 2) + (8 if u % 2 else 0)
                        nc.tensor.matmul(
                            pv0, pt, xn[jt][:, 0:512], start=u == 0, stop=u == nj - 1
                        )
                        nc.tensor.matmul(
                            pv1, pt, xn[jt][:, 512:1024], start=u == 0, stop=u == nj - 1
                        )
                    usb = ubuf.tile([128, H], BF, tag="u", name=f"u{li}")
                    nc.scalar.copy(out=usb[:, 0:512], in_=pv0)
                    nc.scalar.copy(out=usb[:, 512:1024], in_=pv1)
                    for hs in range(8):
                        tp = tpp.tile([128, 128], BF, tag="tp", name=f"tpu{li}_{hs}")
                        nc.tensor.transpose(tp, usb[:, 128 * hs:128 * hs + 128], ident)
                        nc.vector.tensor_copy(out=ut[:, hs, :], in_=tp)
                else:
                    # small j-window: accumulate U^T directly (shorter serial
                    # chain; PE has slack here)
                    pts_list = []
                    for u in range(nj):
                        tp = tpp.tile([128, 128], BF, tag="tp", name=f"tp{li}_{u}")
                        nc.tensor.transpose(tp, p[:, 128 * u:128 * u + 128], ident)
                        pt = ptsp.tile([128, 128], BF, tag="pts", name=f"pt{li}_{u}")
                        nc.vector.tensor_copy(out=pt, in_=tp)
                        pts_list.append(pt)
                    for ht in range(8):
                        up = accp.tile([128, 128], F32, tag="acc", name=f"up{li}_{ht}")
                        for u in range(nj):
                            jt = (li + u // 2) + (8 if u % 2 else 0)
                            nc.tensor.matmul(
                                up,
                                xn[jt][:, 128 * ht:128 * ht + 128],
                                pts_list[u],
                                start=u == 0,
                                stop=u == nj - 1,
                            )
                        nc.vector.tensor_copy(out=ut[:, ht, :], in_=up)
                # out = U Wv^T, then normalize by the softmax denominator
                den = smalls.tile([128, 1], F32, tag="den", name=f"den{li}")
                nc.vector.reduce_sum(den, asum[:, 0:nch], axis=mybir.AxisListType.X)
                rden = smalls.tile([128, 1], F32, tag="rden", name=f"rden{li}")
                nc.vector.reciprocal(rden, den)
                ob = outb.tile([128, 1024], F32, tag="ob", name=f"ob{li}")
                for eh in range(2):
                    po = accp.tile([128, 512], F32, tag="acc", name=f"po{li}_{eh}")
                    for hs in range(8):
                        nc.tensor.matmul(
                            po,
                            ut[:, hs, :],
                            wvT_sb[:, hs, 512 * eh:512 * eh + 512],
                            start=hs == 0,
                            stop=hs == 7,
                        )
                    nc.vector.tensor_scalar_mul(
                        ob[:, 512 * eh:512 * eh + 512], po, rden
                    )
                    nc.sync.dma_start(
                        out=out_ext[li, :, 512 * eh:512 * eh + 512],
                        in_=ob[:, 512 * eh:512 * eh + 512],
                    )

    nc.compile()
    return nc


def _amask(s: int) -> np.ndarray:
    # Additive mask for chunk 0 = [own diagonal tile | partner tile]; the
    # partner tile of slot li is global tile 2li+(1-s): above the diagonal
    # for s=0 (keep), below for s=1 (mask out).
    m = np.zeros((128, 256), dtype=np.float32)
    i = np.arange(128)[:, None]
    j = np.arange(128)[None, :]
    m[:, 0:128] = np.where(j >= i, 0.0, -1e9).astype(np.float32)
    if s == 1:
        m[:, 128:256] = -1e9
    return m


def _perm(s: int) -> np.ndarray:
    own = [2 * u + s for u in range(8)]
    other = [2 * u + 1 - s for u in range(8)]
    return np.array(own + other)


def kernel(input: np.ndarray, w: np.ndarray) -> np.ndarray:
    global LAST_RESULT
    if "nc" not in _CACHE:
        _CACHE["nc"] = _build()
    nc = _CACHE["nc"]

    input = np.ascontiguousarray(input, dtype=np.float32)
    w = np.ascontiguousarray(w, dtype=np.float32)
    wT = np.ascontiguousarray(w.T)                      # [H, 3E]
    wQ = np.ascontiguousarray(w[0:E, :])                # [E, H]
    ident = np.eye(128, dtype=ml_dtypes.bfloat16)

    in_maps = []
    for c in range(8):
        b, s = divmod(c, 2)
        perm = _perm(s)
        xt3 = input[b].T.reshape(H, NT, 128)            # [H, 16, 128]
        xT = np.ascontiguousarray(
            xt3[:, perm, :].reshape(H, N)
        )                                               # [H, N] col-tiles permuted
        xn3 = input[b].reshape(NT, 128, H)
        xn = np.ascontiguousarray(
            xn3[perm].reshape(N, H)
        )                                               # [N, H] row-tiles permuted
        in_maps.append(
            {
                "xT": xT,
                "xn": xn,
                "wQ": wQ,
                "wT": wT,
                "amask": _amask(s),
                "ident": ident,
            }
        )

    trace = bool(int(os.environ.get("KERNEL_TRACE", "0")))
    res = run_bass_kernel_spmd(nc, in_maps, core_ids=list(range(8)), trace=trace)
    LAST_RESULT = res

    out = np.empty((B, N, E), dtype=np.float32)
    for c in range(8):
        b, s = divmod(c, 2)
        o = res.results[c]["out"]                       # [LT, 128, 1024]
        for lt in range(LT):
            r0 = 128 * (2 * lt + s)
            out[b, r0:r0 + 128, :] = o[lt]
    return out


# revision 37
# speedup vs baseline: 1.1700x; 1.1700x over previous
"""Distributed AttentionHead kernel for 8 TRN2 NeuronCores.

Problem: qkv = x @ w.T ; q,k,v = split(qkv); scores[i,j] = k_i.q_j/sqrt(E),
mask keeps j >= i; out = softmax(scores) @ v.   B,N,H,E = 4,2048,1024,1024.

Sharding: core c = 2*b + s handles batch b; it owns the 8 row-tiles
{128*(2*lt+s) : lt in 0..7} (parity interleave => every core's attention
loop has j-extents (16,14,12,...,2) tiles => a single uniform SPMD graph).
Masks that differ between even/odd cores are passed as input *data*,
keeping the graph identical on all cores (required: collectives-free SPMD).

Algebraic restructure (saves ~2x projection FLOPs vs materializing q/v):
  scores = K . (Wq x^T) = (K Wq) . x^T           -> T = K Wq, own rows only
  out    = P (X Wv^T)   = (P X) Wv^T             -> U = P X,  own rows only
so no full-batch q or v is ever computed; every GEMM is own-row-sized
except the (mask-skipped) attention contractions themselves.

Per-core work: k^T proj 2.15 GF + T 2.15 + scores ~2.4 + U ~2.4 + out 2.15
= ~11.3 GF.  Compute dtype: bf16 operands (cast during DMA), fp32 accum.
"""
import os
import sys

sys.path.insert(0, "/opt/trn_rl_repo")

import numpy as np
import ml_dtypes

import concourse.mybir as mybir
from concourse import bacc
from concourse.tile import TileContext
from concourse.bass_utils import run_bass_kernel_spmd

B, N, H, E = 4, 2048, 1024, 1024
O3 = 3 * E
NT = N // 128          # 16 row tiles per batch
LT = 8                 # row tiles owned per core
BF = mybir.dt.bfloat16
F32 = mybir.dt.float32

_CACHE = {}
LAST_RESULT = None


def _build():
    nc = bacc.Bacc("TRN2", target_bir_lowering=False, debug=False, num_devices=8)

    xT_ext = nc.dram_tensor("xT", [H, N], F32, kind="ExternalInput")
    xn_ext = nc.dram_tensor("xn", [N, H], F32, kind="ExternalInput")
    wQ_ext = nc.dram_tensor("wQ", [E, H], F32, kind="ExternalInput")
    wT_ext = nc.dram_tensor("wT", [H, O3], F32, kind="ExternalInput")
    am_ext = nc.dram_tensor("amask", [128, 256], F32, kind="ExternalInput")
    id_ext = nc.dram_tensor("ident", [128, 128], BF, kind="ExternalInput")
    out_ext = nc.dram_tensor("out", [LT, 128, 1024], F32, kind="ExternalOutput")

    xT_r = xT_ext.rearrange("(hs p) n -> p hs n", p=128)
    xn_r = xn_ext.rearrange("(jt p) h -> p jt h", p=128)
    wQ_r = wQ_ext.rearrange("(es p) h -> p es h", p=128)
    wT_r = wT_ext.rearrange("(hs p) o -> p hs o", p=128)

    with TileContext(nc) as tc:
        with (
            tc.tile_pool(name="consts", bufs=1) as consts,
            tc.tile_pool(name="wts", bufs=4) as wts,
            tc.tile_pool(name="bigx", bufs=1) as bigx,
            tc.tile_pool(name="qkv", bufs=1) as qkv,
            tc.tile_pool(name="pbuf", bufs=1) as pbuf,
            tc.tile_pool(name="pts", bufs=6) as ptsp,
            tc.tile_pool(name="ubuf", bufs=3) as ubuf,
            tc.tile_pool(name="utb", bufs=2) as utb,
            tc.tile_pool(name="outb", bufs=2) as outb,
            tc.tile_pool(name="smalls", bufs=3) as smalls,
            tc.tile_pool(name="acc", bufs=4, space="PSUM") as accp,
            tc.tile_pool(name="sc", bufs=2, space="PSUM") as scp,
            tc.tile_pool(name="tp", bufs=2, space="PSUM") as tpp,
        ):
            ident = consts.tile([128, 128], BF)
            nc.sync.dma_start(out=ident, in_=id_ext[:, :])
            am_sb = consts.tile([128, 256], F32)
            nc.sync.dma_start(out=am_sb, in_=am_ext[:, :])

            # Warm-up matmuls: keep the PE busy while the first inputs load so
            # the HAM clock gate reaches 8/8 before real work arrives, instead
            # of paying the cold 1.2 GHz ramp on the first projection.
            wu_sb = consts.tile([128, 512], BF)
            nc.vector.memset(wu_sb, 0.0)
            wu_lhs = consts.tile([128, 128], BF)
            nc.vector.memset(wu_lhs, 0.0)
            wu_ps = accp.tile([128, 512], F32, tag="acc", name="wu_ps")
            for r in range(14):
                nc.tensor.matmul(wu_ps, wu_lhs, wu_sb, start=True, stop=True)

            # k-projection inputs, ordered for earliest possible PE start:
            # first wT chunk first (only 1MB), then xTo slab-by-slab so the
            # first psum group can begin as soon as slab 0 lands.
            wtc_pre = {}
            wtc = wts.tile([128, 8, 256], BF, tag="wtc", name="wtc4")
            nc.gpsimd.dma_start(out=wtc, in_=wT_r[:, :, 256 * 4:256 * 4 + 256])
            wtc_pre[4] = wtc

            xT_sb = bigx.tile([128, 8, N], BF)
            for hs in range(8):
                nc.gpsimd.dma_start(
                    out=xT_sb[:, hs, 0:N // 2], in_=xT_r[:, hs, 0:N // 2]
                )

            for oc in (5, 6, 7):
                wtc = wts.tile([128, 8, 256], BF, tag="wtc", name=f"wtc{oc}")
                nc.gpsimd.dma_start(out=wtc, in_=wT_r[:, :, 256 * oc:256 * oc + 256])
                wtc_pre[oc] = wtc

            wQ_sb = bigx.tile([128, 8, H], BF)
            nc.gpsimd.dma_start(out=wQ_sb, in_=wQ_r)

            nc.gpsimd.dma_start(out=xT_sb[:, :, N // 2:N], in_=xT_r[:, :, N // 2:N])

            xn_sb = qkv.tile([128, NT, H], BF, tag="xn", name="xn_sb")
            nc.gpsimd.dma_start(out=xn_sb, in_=xn_r)
            xn = [xn_sb[:, t, :] for t in range(NT)]

            wvT_sb = bigx.tile([128, 8, E], BF)
            nc.gpsimd.dma_start(out=wvT_sb, in_=wT_r[:, :, 2 * E:3 * E])

            kT = [qkv.tile([128, N // 2], BF, tag=f"kT{e}", name=f"kT{e}") for e in range(8)]
            TT = [qkv.tile([128, N // 2], BF, tag=f"TT{h}", name=f"TT{h}") for h in range(8)]

            # ---------------- k^T projection (own rows) ----------------
            for oc in (4, 5, 6, 7):
                wtc = wtc_pre[oc]
                for n0 in range(0, N // 2, 512):
                    for ot in range(2):
                        et = 2 * (oc - 4) + ot
                        ps = accp.tile([128, 512], F32, tag="acc", name="ps_k")
                        for hs in range(8):
                            nc.tensor.matmul(
                                ps,
                                wtc[:, hs, 128 * ot:128 * ot + 128],
                                xT_sb[:, hs, n0:n0 + 512],
                                start=hs == 0,
                                stop=hs == 7,
                            )
                        nc.scalar.copy(out=kT[et][:, n0:n0 + 512], in_=ps)

            # ---------------- T^T = (K Wq)^T = Wq^T K^T ----------------
            for i0 in range(0, N // 2, 512):
                for ht in range(8):
                    ps = accp.tile([128, 512], F32, tag="acc", name="ps_t")
                    for es in range(8):
                        nc.tensor.matmul(
                            ps,
                            wQ_sb[:, es, 128 * ht:128 * ht + 128],
                            kT[es][:, i0:i0 + 512],
                            start=es == 0,
                            stop=es == 7,
                        )
                    nc.vector.tensor_copy(out=TT[ht][:, i0:i0 + 512], in_=ps)

            # ---------------- attention ----------------
            for li in range(LT):
                nch = 8 - li          # 256-wide score chunks
                nj = NT - 2 * li      # 128-wide j tiles
                p = pbuf.tile([128, 256 * nch], BF, tag=f"p{li}", name=f"p{li}")
                asum = smalls.tile([128, 8], F32, tag="asum", name=f"asum{li}")
                for ch in range(nch):
                    g = li + ch
                    ps = scp.tile([128, 256], F32, tag="sc", name=f"ps_s{li}_{ch}")
                    for hs in range(8):
                        rhs = xT_sb[:, hs, :].rearrange(
                            "p (two g c) -> p two g c", two=2, c=128
                        )[:, :, g, :]
                        nc.tensor.matmul(
                            ps,
                            TT[hs][:, 128 * li:128 * li + 128],
                            rhs,
                            start=hs == 0,
                            stop=hs == 7,
                        )
                    if ch == 0:
                        nc.vector.tensor_add(ps, ps, am_sb)
                    nc.scalar.activation(
                        out=p[:, 256 * ch:256 * ch + 256],
                        in_=ps,
                        func=mybir.ActivationFunctionType.Exp,
                        scale=float(1.0 / np.sqrt(E)),
                        accum_out=asum[:, ch:ch + 1],
                    )
                ut = utb.tile([128, 8, 128], BF, tag="ut", name=f"ut{li}")
                if li < 5:
                    # U = P X  (f32 accum in PSUM, bf16 out), then transpose U
                    pv0 = accp.tile([128, 512], F32, tag="acc", name=f"pv0_{li}")
                    pv1 = accp.tile([128, 512], F32, tag="acc", name=f"pv1_{li}")
                    for u in range(nj):
                        tp = tpp.tile([128, 128], BF, tag="tp", name=f"tp{li}_{u}")
                        nc.tensor.transpose(tp, p[:, 128 * u:128 * u + 128], ident)
                        pt = ptsp.tile([128, 128], BF, tag="pts", name=f"pt{li}_{u}")
                        nc.vector.tensor_copy(out=pt, in_=tp)
                        jt = (li + u // 2) + (8 if u % 2 else 0)
                        nc.tensor.matmul(
                            pv0, pt, xn[jt][:, 0:512], start=u == 0, stop=u == nj - 1
                        )
                        nc.tensor.matmul(
                            pv1, pt, xn[jt][:, 512:1024], start=u == 0, stop=u == nj - 1
                        )
                    usb = ubuf.tile([128, H], BF, tag="u", name=f"u{li}")
                    nc.scalar.copy(out=usb[:, 0:512], in_=pv0)
                    nc.scalar.copy(out=usb[:, 512:1024], in_=pv1)
                    for hs in range(8):
                        tp = tpp.tile([128, 128], BF, tag="tp", name=f"tpu{li}_{hs}")
                        nc.tensor.transpose(tp, usb[:, 128 * hs:128 * hs + 128], ident)
                        nc.vector.tensor_copy(out=ut[:, hs, :], in_=tp)
                else:
                    # small j-window: accumulate U^T directly (shorter serial
                    # chain; PE has slack here)
                    pts_list = []
                    for u in range(nj):
                        tp = tpp.tile([128, 128], BF, tag="tp", name=f"tp{li}_{u}")
                        nc.tensor.transpose(tp, p[:, 128 * u:128 * u + 128], ident)
                        pt = ptsp.tile([128, 128], BF, tag="pts", name=f"pt{li}_{u}")
                        nc.vector.tensor_copy(out=pt, in_=tp)
                        pts_list.append(pt)
                    for ht in range(8):
                        up = accp.tile([128, 128], F32, tag="acc", name=f"up{li}_{ht}")
                        for u in range(nj):
                            jt = (li + u // 2) + (8 if u % 2 else 0)
                            nc.tensor.matmul(
                                up,
                                xn[jt][:, 128 * ht:128 * ht + 128],
                                pts_list[u],
                                start=u == 0,
                                stop=u == nj - 1,
                            )
                        nc.vector.tensor_copy(out=ut[:, ht, :], in_=up)
                # out = U Wv^T, then normalize by the softmax denominator
                den = smalls.tile([128, 1], F32, tag="den", name=f"den{li}")
                nc.vector.reduce_sum(den, asum[:, 0:nch], axis=mybir.AxisListType.X)
                rden = smalls.tile([128, 1], F32, tag="rden", name=f"rden{li}")
                nc.vector.reciprocal(rden, den)
                ob = outb.tile([128, 1024], F32, tag="ob", name=f"ob{li}")
                for eh in range(2):
                    po = accp.tile([128, 512], F32, tag="acc", name=f"po{li}_{eh}")
                    for hs in range(8):
                        nc.tensor.matmul(
                            po,
                            ut[:, hs, :],
                            wvT_sb[:, hs, 512 * eh:512 * eh + 512],
                            start=hs == 0,
                            stop=hs == 7,
                        )
                    nc.vector.tensor_scalar_mul(
                        ob[:, 512 * eh:512 * eh + 512], po, rden
                    )
                    nc.sync.dma_start(
                        out=out_ext[li, :, 512 * eh:512 * eh + 512],
                        in_=ob[:, 512 * eh:512 * eh + 512],
                    )

    nc.compile()
    return nc


def _amask(s: int) -> np.ndarray:
    # Additive mask for chunk 0 = [own diagonal tile | partner tile]; the
    # partner tile of slot li is global tile 2li+(1-s): above the diagonal
    # for s=0 (keep), below for s=1 (mask out).
    m = np.zeros((128, 256), dtype=np.float32)
    i = np.arange(128)[:, None]
    j = np.arange(128)[None, :]
    m[:, 0:128] = np.where(j >= i, 0.0, -1e9).astype(np.float32)
    if s == 1:
        m[:, 128:256] = -1e9
    return m


def _perm(s: int) -> np.ndarray:
    own = [2 * u + s for u in range(8)]
    other = [2 * u + 1 - s for u in range(8)]
    return np.array(own + other)


def kernel(input: np.ndarray, w: np.ndarray) -> np.ndarray:
    global LAST_RESULT
    if "nc" not in _CACHE:
        _CACHE["nc"] = _build()
    nc = _CACHE["nc"]

    input = np.ascontiguousarray(input, dtype=np.float32)
    w = np.ascontiguousarray(w, dtype=np.float32)
    wT = np.ascontiguousarray(w.T)                      # [H, 3E]
    wQ = np.ascontiguousarray(w[0:E, :])                # [E, H]
    ident = np.eye(128, dtype=ml_dtypes.bfloat16)

    in_maps = []
    for c in range(8):
        b, s = divmod(c, 2)
        perm = _perm(s)
        xt3 = input[b].T.reshape(H, NT, 128)            # [H, 16, 128]
        xT = np.ascontiguousarray(
            xt3[:, perm, :].reshape(H, N)
        )                                               # [H, N] col-tiles permuted
        xn3 = input[b].reshape(NT, 128, H)
        xn = np.ascontiguousarray(
            xn3[perm].reshape(N, H)
        )                                               # [N, H] row-tiles permuted
        in_maps.append(
            {
                "xT": xT,
                "xn": xn,
                "wQ": wQ,
                "wT": wT,
                "amask": _amask(s),
                "ident": ident,
            }
        )

    trace = bool(int(os.environ.get("KERNEL_TRACE", "0")))
    res = run_bass_kernel_spmd(nc, in_maps, core_ids=list(range(8)), trace=trace)
    LAST_RESULT = res

    out = np.empty((B, N, E), dtype=np.float32)
    for c in range(8):
        b, s = divmod(c, 2)
        o = res.results[c]["out"]                       # [LT, 128, 1024]
        for lt in range(LT):
            r0 = 128 * (2 * lt + s)
            out[b, r0:r0 + 128, :] = o[lt]
    return out


# revision 38
# speedup vs baseline: 1.1772x; 1.0061x over previous
"""Distributed AttentionHead kernel for 8 TRN2 NeuronCores.

Problem: qkv = x @ w.T ; q,k,v = split(qkv); scores[i,j] = k_i.q_j/sqrt(E),
mask keeps j >= i; out = softmax(scores) @ v.   B,N,H,E = 4,2048,1024,1024.

Sharding: core c = 2*b + s handles batch b; it owns the 8 row-tiles
{128*(2*lt+s) : lt in 0..7} (parity interleave => every core's attention
loop has j-extents (16,14,12,...,2) tiles => a single uniform SPMD graph).
Masks that differ between even/odd cores are passed as input *data*,
keeping the graph identical on all cores (required: collectives-free SPMD).

Algebraic restructure (saves ~2x projection FLOPs vs materializing q/v):
  scores = K . (Wq x^T) = (K Wq) . x^T           -> T = K Wq, own rows only
  out    = P (X Wv^T)   = (P X) Wv^T             -> U = P X,  own rows only
so no full-batch q or v is ever computed; every GEMM is own-row-sized
except the (mask-skipped) attention contractions themselves.

Per-core work: k^T proj 2.15 GF + T 2.15 + scores ~2.4 + U ~2.4 + out 2.15
= ~11.3 GF.  Compute dtype: bf16 operands (cast during DMA), fp32 accum.
"""
import os
import sys

sys.path.insert(0, "/opt/trn_rl_repo")

import numpy as np
import ml_dtypes

import concourse.mybir as mybir
from concourse import bacc
from concourse.tile import TileContext
from concourse.bass_utils import run_bass_kernel_spmd

B, N, H, E = 4, 2048, 1024, 1024
O3 = 3 * E
NT = N // 128          # 16 row tiles per batch
LT = 8                 # row tiles owned per core
BF = mybir.dt.bfloat16
F32 = mybir.dt.float32

_CACHE = {}
LAST_RESULT = None


def _build():
    nc = bacc.Bacc("TRN2", target_bir_lowering=False, debug=False, num_devices=8)

    xT_ext = nc.dram_tensor("xT", [H, N], F32, kind="ExternalInput")
    xn_ext = nc.dram_tensor("xn", [N, H], F32, kind="ExternalInput")
    wQ_ext = nc.dram_tensor("wQ", [E, H], F32, kind="ExternalInput")
    wT_ext = nc.dram_tensor("wT", [H, O3], F32, kind="ExternalInput")
    am_ext = nc.dram_tensor("amask", [128, 256], F32, kind="ExternalInput")
    id_ext = nc.dram_tensor("ident", [128, 128], BF, kind="ExternalInput")
    out_ext = nc.dram_tensor("out", [LT, 128, 1024], F32, kind="ExternalOutput")

    xT_r = xT_ext.rearrange("(hs p) n -> p hs n", p=128)
    xn_r = xn_ext.rearrange("(jt p) h -> p jt h", p=128)
    wQ_r = wQ_ext.rearrange("(es p) h -> p es h", p=128)
    wT_r = wT_ext.rearrange("(hs p) o -> p hs o", p=128)

    with TileContext(nc) as tc:
        with (
            tc.tile_pool(name="consts", bufs=1) as consts,
            tc.tile_pool(name="wts", bufs=4) as wts,
            tc.tile_pool(name="bigx", bufs=1) as bigx,
            tc.tile_pool(name="qkv", bufs=1) as qkv,
            tc.tile_pool(name="pbuf", bufs=1) as pbuf,
            tc.tile_pool(name="pts", bufs=6) as ptsp,
            tc.tile_pool(name="ubuf", bufs=3) as ubuf,
            tc.tile_pool(name="utb", bufs=2) as utb,
            tc.tile_pool(name="outb", bufs=2) as outb,
            tc.tile_pool(name="smalls", bufs=3) as smalls,
            tc.tile_pool(name="acc", bufs=4, space="PSUM") as accp,
            tc.tile_pool(name="sc", bufs=2, space="PSUM") as scp,
            tc.tile_pool(name="tp", bufs=2, space="PSUM") as tpp,
        ):
            ident = consts.tile([128, 128], BF)
            nc.sync.dma_start(out=ident, in_=id_ext[:, :])
            am_sb = consts.tile([128, 256], F32)
            nc.sync.dma_start(out=am_sb, in_=am_ext[:, :])

            # Warm-up matmuls: keep the PE busy while the first inputs load so
            # the HAM clock gate reaches 8/8 before real work arrives, instead
            # of paying the cold 1.2 GHz ramp on the first projection.
            wu_sb = consts.tile([128, 512], BF)
            nc.vector.memset(wu_sb, 0.0)
            wu_lhs = consts.tile([128, 128], BF)
            nc.vector.memset(wu_lhs, 0.0)
            wu_ps = accp.tile([128, 512], F32, tag="acc", name="wu_ps")
            for r in range(18):
                nc.tensor.matmul(wu_ps, wu_lhs, wu_sb, start=True, stop=True)

            # k-projection inputs, ordered for earliest possible PE start:
            # first wT chunk first (only 1MB), then xTo slab-by-slab so the
            # first psum group can begin as soon as slab 0 lands.
            wtc_pre = {}
            wtc = wts.tile([128, 8, 256], BF, tag="wtc", name="wtc4")
            nc.gpsimd.dma_start(out=wtc, in_=wT_r[:, :, 256 * 4:256 * 4 + 256])
            wtc_pre[4] = wtc

            xT_sb = bigx.tile([128, 8, N], BF)
            for hs in range(8):
                nc.gpsimd.dma_start(
                    out=xT_sb[:, hs, 0:N // 2], in_=xT_r[:, hs, 0:N // 2]
                )

            for oc in (5, 6, 7):
                wtc = wts.tile([128, 8, 256], BF, tag="wtc", name=f"wtc{oc}")
                nc.gpsimd.dma_start(out=wtc, in_=wT_r[:, :, 256 * oc:256 * oc + 256])
                wtc_pre[oc] = wtc

            wQ_sb = bigx.tile([128, 8, H], BF)
            nc.gpsimd.dma_start(out=wQ_sb, in_=wQ_r)

            nc.gpsimd.dma_start(out=xT_sb[:, :, N // 2:N], in_=xT_r[:, :, N // 2:N])

            xn_sb = qkv.tile([128, NT, H], BF, tag="xn", name="xn_sb")
            nc.gpsimd.dma_start(out=xn_sb, in_=xn_r)
            xn = [xn_sb[:, t, :] for t in range(NT)]

            wvT_sb = bigx.tile([128, 8, E], BF)
            nc.gpsimd.dma_start(out=wvT_sb, in_=wT_r[:, :, 2 * E:3 * E])

            kT = [qkv.tile([128, N // 2], BF, tag=f"kT{e}", name=f"kT{e}") for e in range(8)]
            TT = [qkv.tile([128, N // 2], BF, tag=f"TT{h}", name=f"TT{h}") for h in range(8)]

            # ---------------- k^T projection (own rows) ----------------
            for oc in (4, 5, 6, 7):
                wtc = wtc_pre[oc]
                for n0 in range(0, N // 2, 512):
                    for ot in range(2):
                        et = 2 * (oc - 4) + ot
                        ps = accp.tile([128, 512], F32, tag="acc", name="ps_k")
                        for hs in range(8):
                            nc.tensor.matmul(
                                ps,
                                wtc[:, hs, 128 * ot:128 * ot + 128],
                                xT_sb[:, hs, n0:n0 + 512],
                                start=hs == 0,
                                stop=hs == 7,
                            )
                        nc.scalar.copy(out=kT[et][:, n0:n0 + 512], in_=ps)

            # ---------------- T^T = (K Wq)^T = Wq^T K^T ----------------
            for i0 in range(0, N // 2, 512):
                for ht in range(8):
                    ps = accp.tile([128, 512], F32, tag="acc", name="ps_t")
                    for es in range(8):
                        nc.tensor.matmul(
                            ps,
                            wQ_sb[:, es, 128 * ht:128 * ht + 128],
                            kT[es][:, i0:i0 + 512],
                            start=es == 0,
                            stop=es == 7,
                        )
                    nc.vector.tensor_copy(out=TT[ht][:, i0:i0 + 512], in_=ps)

            # ---------------- attention ----------------
            for li in range(LT):
                nch = 8 - li          # 256-wide score chunks
                nj = NT - 2 * li      # 128-wide j tiles
                p = pbuf.tile([128, 256 * nch], BF, tag=f"p{li}", name=f"p{li}")
                asum = smalls.tile([128, 8], F32, tag="asum", name=f"asum{li}")
                for ch in range(nch):
                    g = li + ch
                    ps = scp.tile([128, 256], F32, tag="sc", name=f"ps_s{li}_{ch}")
                    for hs in range(8):
                        rhs = xT_sb[:, hs, :].rearrange(
                            "p (two g c) -> p two g c", two=2, c=128
                        )[:, :, g, :]
                        nc.tensor.matmul(
                            ps,
                            TT[hs][:, 128 * li:128 * li + 128],
                            rhs,
                            start=hs == 0,
                            stop=hs == 7,
                        )
                    if ch == 0:
                        nc.vector.tensor_add(ps, ps, am_sb)
                    nc.scalar.activation(
                        out=p[:, 256 * ch:256 * ch + 256],
                        in_=ps,
                        func=mybir.ActivationFunctionType.Exp,
                        scale=float(1.0 / np.sqrt(E)),
                        accum_out=asum[:, ch:ch + 1],
                    )
                ut = utb.tile([128, 8, 128], BF, tag="ut", name=f"ut{li}")
                if li < 5:
                    # U = P X  (f32 accum in PSUM, bf16 out), then transpose U
                    pv0 = accp.tile([128, 512], F32, tag="acc", name=f"pv0_{li}")
                    pv1 = accp.tile([128, 512], F32, tag="acc", name=f"pv1_{li}")
                    for u in range(nj):
                        tp = tpp.tile([128, 128], BF, tag="tp", name=f"tp{li}_{u}")
                        nc.tensor.transpose(tp, p[:, 128 * u:128 * u + 128], ident)
                        pt = ptsp.tile([128, 128], BF, tag="pts", name=f"pt{li}_{u}")
                        nc.vector.tensor_copy(out=pt, in_=tp)
                        jt = (li + u // 2) + (8 if u % 2 else 0)
                        nc.tensor.matmul(
                            pv0, pt, xn[jt][:, 0:512], start=u == 0, stop=u == nj - 1
                        )
                        nc.tensor.matmul(
                            pv1, pt, xn[jt][:, 512:1024], start=u == 0, stop=u == nj - 1
                        )
                    usb = ubuf.tile([128, H], BF, tag="u", name=f"u{li}")
                    nc.scalar.copy(out=usb[:, 0:512], in_=pv0)
                    nc.scalar.copy(out=usb[:, 512:1024], in_=pv1)
                    for hs in range(8):
                        tp = tpp.tile([128, 128], BF, tag="tp", name=f"tpu{li}_{hs}")
                        nc.tensor.transpose(tp, usb[:, 128 * hs:128 * hs + 128], ident)
                        nc.vector.tensor_copy(out=ut[:, hs, :], in_=tp)
                else:
                    # small j-window: accumulate U^T directly (shorter serial
                    # chain; PE has slack here)
                    pts_list = []
                    for u in range(nj):
                        tp = tpp.tile([128, 128], BF, tag="tp", name=f"tp{li}_{u}")
                        nc.tensor.transpose(tp, p[:, 128 * u:128 * u + 128], ident)
                        pt = ptsp.tile([128, 128], BF, tag="pts", name=f"pt{li}_{u}")
                        nc.vector.tensor_copy(out=pt, in_=tp)
                        pts_list.append(pt)
                    for ht in range(8):
                        up = accp.tile([128, 128], F32, tag="acc", name=f"up{li}_{ht}")
                        for u in range(nj):
                            jt = (li + u // 2) + (8 if u % 2 else 0)
                            nc.tensor.matmul(
                                up,
                                xn[jt][:, 128 * ht:128 * ht + 128],
                                pts_list[u],
                                start=u == 0,
                                stop=u == nj - 1,
                            )
                        nc.vector.tensor_copy(out=ut[:, ht, :], in_=up)
                # out = U Wv^T, then normalize by the softmax denominator
                den = smalls.tile([128, 1], F32, tag="den", name=f"den{li}")
                nc.vector.reduce_sum(den, asum[:, 0:nch], axis=mybir.AxisListType.X)
                rden = smalls.tile([128, 1], F32, tag="rden", name=f"rden{li}")
                nc.vector.reciprocal(rden, den)
                ob = outb.tile([128, 1024], F32, tag="ob", name=f"ob{li}")
                for eh in range(2):
                    po = accp.tile([128, 512], F32, tag="acc", name=f"po{li}_{eh}")
                    for hs in range(8):
                        nc.tensor.matmul(
                            po,
                            ut[:, hs, :],
                            wvT_sb[:, hs, 512 * eh:512 * eh + 512],
                            start=hs == 0,
                            stop=hs == 7,
                        )
                    nc.vector.tensor_scalar_mul(
                        ob[:, 512 * eh:512 * eh + 512], po, rden
                    )
                    nc.sync.dma_start(
                        out=out_ext[li, :, 512 * eh:512 * eh + 512],
                        in_=ob[:, 512 * eh:512 * eh + 512],
                    )

    nc.compile()
    return nc


def _amask(s: int) -> np.ndarray:
    # Additive mask for chunk 0 = [own diagonal tile | partner tile]; the
    # partner tile of slot li is global tile 2li+(1-s): above the diagonal
    # for s=0 (keep), below for s=1 (mask out).
    m = np.zeros((128, 256), dtype=np.float32)
    i = np.arange(128)[:, None]
    j = np.arange(128)[None, :]
    m[:, 0:128] = np.where(j >= i, 0.0, -1e9).astype(np.float32)
    if s == 1:
        m[:, 128:256] = -1e9
    return m


def _perm(s: int) -> np.ndarray:
    own = [2 * u + s for u in range(8)]
    other = [2 * u + 1 - s for u in range(8)]
    return np.array(own + other)


def kernel(input: np.ndarray, w: np.ndarray) -> np.ndarray:
    global LAST_RESULT
    if "nc" not in _CACHE:
        _CACHE["nc"] = _build()
    nc = _CACHE["nc"]

    input = np.ascontiguousarray(input, dtype=np.float32)
    w = np.ascontiguousarray(w, dtype=np.float32)
    wT = np.ascontiguousarray(w.T)                      # [H, 3E]
    wQ = np.ascontiguousarray(w[0:E, :])                # [E, H]
    ident = np.eye(128, dtype=ml_dtypes.bfloat16)

    in_maps = []
    for c in range(8):
        b, s = divmod(c, 2)
        perm = _perm(s)
        xt3 = input[b].T.reshape(H, NT, 128)            # [H, 16, 128]
        xT = np.ascontiguousarray(
            xt3[:, perm, :].reshape(H, N)
        )                                               # [H, N] col-tiles permuted
        xn3 = input[b].reshape(NT, 128, H)
        xn = np.ascontiguousarray(
            xn3[perm].reshape(N, H)
        )                                               # [N, H] row-tiles permuted
        in_maps.append(
            {
                "xT": xT,
                "xn": xn,
                "wQ": wQ,
                "wT": wT,
                "amask": _amask(s),
                "ident": ident,
            }
        )

    trace = bool(int(os.environ.get("KERNEL_TRACE", "0")))
    res = run_bass_kernel_spmd(nc, in_maps, core_ids=list(range(8)), trace=trace)
    LAST_RESULT = res

    out = np.empty((B, N, E), dtype=np.float32)
    for c in range(8):
        b, s = divmod(c, 2)
        o = res.results[c]["out"]                       # [LT, 128, 1024]
        for lt in range(LT):
            r0 = 128 * (2 * lt + s)
            out[b, r0:r0 + 128, :] = o[lt]
    return out
